# revision 10
# baseline (speedup 1.0000x reference)
"""Bass/Trainium2 kernel for nn_AvgPoolBackbone (segment_reduce).

Computes, for each batch row b of x [B, S, D]:
    eff = S if idx[b] == -1 else idx[b]
    out[b] = mean(x[b, :eff], axis=0)   (zeros when eff <= 0)

Strategy
--------
Pure data parallel over 8 NeuronCores (16 batches each).  On the host we
fold the prefix mask AND the 1/eff_len scaling into a single f32 matrix
`maskt` so the device does no division and no control flow.

Per batch, x[b] ([2048, 256] f32, 2 MiB) is viewed as [128, 16*256]:
partition p holds the 16 consecutive sequence rows p*16..p*16+15 — one
contiguous 16 KiB DRAM run per partition, which keeps the DMA descriptors
large.  The masked mean is then 16 PSUM-accumulated TensorE matmuls

    psum[1, D] += maskt[:, col].T @ x_view[:, j*D:(j+1)*D]

where maskt[p, col] = mask[b, p*16 + j] / eff_len[b].  Operands are
bitcast to float32r, which streams one PSUM row per cycle (4x faster
than the two-pass fp32 path) at N=256.  TensorE does the masking and the
cross-partition reduction in one instruction; the kernel is
HBM-bandwidth bound.
"""

import numpy as np

import concourse.bass as bass
import concourse.tile as tile
from concourse import bacc, mybir
from concourse import bass_utils

F32 = mybir.dt.float32
F32R = mybir.dt.float32r

# Problem config (hardcoded per the harness contract).
B, S, D = 128, 2048, 256
N_CORES = 8
BL = B // N_CORES  # batches per core
P = 128            # SBUF partitions
CHUNK_B = 2        # batches loaded per DMA


def build_kernel(bl=BL, s=S, d=D, chunk_b=CHUNK_B, f32r=False, dve_mod=2, bufs=4):
    """Build + compile the single-core Bass module (same NEFF on all cores).

    Batches alternate between two engines to halve the per-engine load
    while keeping exact fp32: batches with b % dve_mod == 0 run a DVE
    fused multiply-accumulate chain (then one PE ones-matmul folds the
    [128, d] partials across partitions); the other batches run 16
    PSUM-accumulated PE matmuls.  PSUM->SBUF result copies go to the
    otherwise idle ScalarE.  With f32r=True everything instead runs on
    PE in reduced-precision float32r (dve_mod ignored).
    """
    j = s // P  # seq rows per partition (16 at full size)
    mmdt = F32R if f32r else F32
    if f32r:
        dve_mod = 0
    nc = bacc.Bacc("TRN2", target_bir_lowering=False, debug=False)
    x = nc.dram_tensor("x", (bl, s, d), mmdt, kind="ExternalInput")
    maskt = nc.dram_tensor("maskt", (P, bl * j), mmdt, kind="ExternalInput")
    out = nc.dram_tensor("out", (1, bl * d), F32, kind="ExternalOutput")

    def is_dve(b):
        return dve_mod > 0 and b % dve_mod == 0

    jh = j // 2 if j % 2 == 0 else j  # j-slices per half-batch DMA
    nh = j // jh                      # DMA halves per batch

    with tile.TileContext(nc) as tc:
        with (
            tc.tile_pool(name="xp", bufs=bufs) as xp,
            tc.tile_pool(name="mp", bufs=1) as mp,
            tc.tile_pool(name="op", bufs=1) as op,
            tc.tile_pool(name="ap", bufs=3) as apool,
            tc.tile_pool(name="ps", bufs=8, space=bass.MemorySpace.PSUM) as ps,
        ):
            m_t = mp.tile([P, bl * j], mmdt)
            nc.sync.dma_start(m_t[:], maskt.ap())
            ones_t = None
            if dve_mod > 0:
                ones_t = mp.tile([P, 1], F32)
                nc.vector.memset(ones_t[:], 1.0)
            o_t = op.tile([1, bl * d], F32)
            xv = x.ap().rearrange("b (p h k) d -> p b h (k d)", p=P, h=nh)
            for b in range(bl):
                # one 1 MiB DMA per half-batch, all on the sync HWDGE ring
                # in consumption order; each lands as [P, jh*d] with
                # contiguous jh*d*4-byte DRAM runs per partition
                halves = []
                for h in range(nh):
                    x_t = xp.tile([P, jh * d], mmdt, tag=f"x{h}")
                    nc.sync.dma_start(x_t[:], xv[:, b, h])
                    halves.append(x_t)

                def slices(ji):
                    col = b * j + ji
                    xs = halves[ji // jh][:, (ji % jh) * d : (ji % jh + 1) * d]
                    return xs, m_t[:, col : col + 1]

                acc = ps.tile([1, d], F32)
                if is_dve(b):
                    acc_sb = apool.tile([P, d], F32)
                    for ji in range(j):
                        xs, mcol = slices(ji)
                        if ji == 0:
                            nc.vector.tensor_scalar_mul(acc_sb[:], xs, mcol)
                        else:
                            nc.vector.scalar_tensor_tensor(
                                acc_sb[:],
                                xs,
                                mcol,
                                acc_sb[:],
                                mybir.AluOpType.mult,
                                mybir.AluOpType.add,
                            )
                    nc.tensor.matmul(
                        acc[:], ones_t[:], acc_sb[:], start=True, stop=True
                    )
                else:
                    for ji in range(j):
                        xs, mcol = slices(ji)
                        nc.tensor.matmul(
                            acc[:],
                            mcol,
                            xs,
                            start=(ji == 0),
                            stop=(ji == j - 1),
                        )
                nc.scalar.copy(o_t[:, b * d : (b + 1) * d], acc[:])
            nc.sync.dma_start(out.ap(), o_t[:])

    nc.compile()
    return nc


def make_host_inputs(x, start_padding_indices, n_cores=N_CORES, bl=BL, s=S, d=D):
    """Shard x and build the per-core scaled mask matrices.

    maskt[p, b*j + ji] = (p*j + ji < eff[b]) / max(eff[b], 1)
    """
    x = np.ascontiguousarray(np.asarray(x, dtype=np.float32))
    idx = np.asarray(start_padding_indices).astype(np.int64)
    j = s // P
    eff = np.where(idx == -1, s, idx).astype(np.int64)  # [B]
    scale = 1.0 / np.maximum(eff, 1).astype(np.float64)
    mask = (np.arange(s)[None, :] < eff[:, None]) * scale[:, None]  # [B, S] f64
    mask = mask.astype(np.float32)
    # [B, S] -> [B, P, j] (s-major within partition) -> cores pack [P, bl*j]
    mask_pj = mask.reshape(-1, P, j)  # [B, P, j]
    in_maps = []
    for c in range(n_cores):
        mb = mask_pj[c * bl : (c + 1) * bl]  # [bl, P, j]
        maskt = np.ascontiguousarray(mb.transpose(1, 0, 2).reshape(P, bl * j))
        in_maps.append(
            {
                "x": np.ascontiguousarray(x[c * bl : (c + 1) * bl]),
                "maskt": maskt,
            }
        )
    return in_maps


_CACHED_NC = None


def _get_nc():
    global _CACHED_NC
    if _CACHED_NC is None:
        _CACHED_NC = build_kernel()
    return _CACHED_NC


def run(x, start_padding_indices, trace=False):
    """Run on all 8 cores; returns (out [B, D] f32, BassKernelResults)."""
    nc = _get_nc()
    in_maps = make_host_inputs(x, start_padding_indices)
    res = bass_utils.run_bass_kernel_spmd(
        nc, in_maps, core_ids=list(range(N_CORES)), trace=trace
    )
    outs = [r["out"].reshape(BL, D) for r in res.results]
    return np.concatenate(outs, axis=0), res


def kernel(x, start_padding_indices):
    out, _ = run(x, start_padding_indices, trace=False)
    return out


# revision 11
# speedup vs baseline: 1.1377x; 1.1377x over previous
"""Bass/Trainium2 kernel for nn_AvgPoolBackbone (segment_reduce).

Computes, for each batch row b of x [B, S, D]:
    eff = S if idx[b] == -1 else idx[b]
    out[b] = mean(x[b, :eff], axis=0)   (zeros when eff <= 0)

Strategy
--------
Pure data parallel over 8 NeuronCores (16 batches each).  On the host we
fold the prefix mask AND the 1/eff_len scaling into a single f32 matrix
`maskt` so the device does no division and no control flow.

Per batch, x[b] ([2048, 256] f32, 2 MiB) is viewed as [128, 16*256]:
partition p holds the 16 consecutive sequence rows p*16..p*16+15 — one
contiguous 16 KiB DRAM run per partition, which keeps the DMA descriptors
large.  The masked mean is then 16 PSUM-accumulated TensorE matmuls

    psum[1, D] += maskt[:, col].T @ x_view[:, j*D:(j+1)*D]

where maskt[p, col] = mask[b, p*16 + j] / eff_len[b].  Operands are
bitcast to float32r, which streams one PSUM row per cycle (4x faster
than the two-pass fp32 path) at N=256.  TensorE does the masking and the
cross-partition reduction in one instruction; the kernel is
HBM-bandwidth bound.
"""

import numpy as np

import concourse.bass as bass
import concourse.tile as tile
from concourse import bacc, mybir
from concourse import bass_utils

F32 = mybir.dt.float32
F32R = mybir.dt.float32r

# Problem config (hardcoded per the harness contract).
B, S, D = 128, 2048, 256
N_CORES = 8
BL = B // N_CORES  # batches per core
P = 128            # SBUF partitions
CHUNK_B = 2        # batches loaded per DMA


def build_kernel(bl=BL, s=S, d=D, chunk_b=CHUNK_B, f32r=False, dve_mod=2, bufs=4):
    """Build + compile the single-core Bass module (same NEFF on all cores).

    Batches alternate between two engines to halve the per-engine load
    while keeping exact fp32: batches with b % dve_mod == 0 run a DVE
    fused multiply-accumulate chain (then one PE ones-matmul folds the
    [128, d] partials across partitions); the other batches run 16
    PSUM-accumulated PE matmuls.  PSUM->SBUF result copies go to the
    otherwise idle ScalarE.  With f32r=True everything instead runs on
    PE in reduced-precision float32r (dve_mod ignored).
    """
    j = s // P  # seq rows per partition (16 at full size)
    mmdt = F32R if f32r else F32
    if f32r:
        dve_mod = 0
    nc = bacc.Bacc("TRN2", target_bir_lowering=False, debug=False)
    x = nc.dram_tensor("x", (bl, s, d), mmdt, kind="ExternalInput")
    maskt = nc.dram_tensor("maskt", (P, bl * j), mmdt, kind="ExternalInput")
    out = nc.dram_tensor("out", (1, bl * d), F32, kind="ExternalOutput")

    def is_dve(b):
        # odd batches on DVE so a PE batch leads the instruction stream
        return dve_mod > 0 and b % dve_mod == 1

    with tile.TileContext(nc) as tc:
        with (
            tc.tile_pool(name="xp", bufs=bufs) as xp,
            tc.tile_pool(name="mp", bufs=1) as mp,
            tc.tile_pool(name="op", bufs=1) as op,
            tc.tile_pool(name="ap", bufs=6) as apool,
            tc.tile_pool(name="ps", bufs=8, space=bass.MemorySpace.PSUM) as ps,
        ):
            m_t = mp.tile([P, bl * j], mmdt)
            nc.sync.dma_start(m_t[:], maskt.ap())
            ones_t = None
            if dve_mod > 0:
                ones_t = mp.tile([P, 1], F32)
                nc.vector.memset(ones_t[:], 1.0)
            o_t = op.tile([1, bl * d], F32)
            xv = x.ap().rearrange("b (p k) d -> p b (k d)", p=P)

            def emit_fold(b, acc_sb):
                # fold the DVE partials of batch b across partitions; the
                # call site defers this until PE has other work queued
                acc = ps.tile([1, d], F32)
                nc.tensor.matmul(
                    acc[:], ones_t[:], acc_sb[:], start=True, stop=True
                )
                nc.scalar.copy(o_t[:, b * d : (b + 1) * d], acc[:])

            pending = None  # (batch, acc_sb) awaiting its fold matmul
            for b in range(bl):
                # one 2 MiB DMA per batch on the sync HWDGE ring, in
                # consumption order; lands as [P, j*d] with one contiguous
                # 16 KiB DRAM run per partition
                x_t = xp.tile([P, j * d], mmdt)
                nc.sync.dma_start(x_t[:], xv[:, b])
                if is_dve(b):
                    acc_sb = apool.tile([P, d], F32)
                    for ji in range(j):
                        xs = x_t[:, ji * d : (ji + 1) * d]
                        mcol = m_t[:, b * j + ji : b * j + ji + 1]
                        if ji == 0:
                            nc.vector.tensor_scalar_mul(acc_sb[:], xs, mcol)
                        else:
                            nc.vector.scalar_tensor_tensor(
                                acc_sb[:],
                                xs,
                                mcol,
                                acc_sb[:],
                                mybir.AluOpType.mult,
                                mybir.AluOpType.add,
                            )
                    pending = (b, acc_sb)
                else:
                    acc = ps.tile([1, d], F32)
                    for ji in range(j):
                        nc.tensor.matmul(
                            acc[:],
                            m_t[:, b * j + ji : b * j + ji + 1],
                            x_t[:, ji * d : (ji + 1) * d],
                            start=(ji == 0),
                            stop=(ji == j - 1),
                        )
                    nc.scalar.copy(o_t[:, b * d : (b + 1) * d], acc[:])
                    if pending is not None:
                        emit_fold(*pending)
                        pending = None
            if pending is not None:
                emit_fold(*pending)
            nc.sync.dma_start(out.ap(), o_t[:])

    nc.compile()
    return nc


def make_host_inputs(x, start_padding_indices, n_cores=N_CORES, bl=BL, s=S, d=D):
    """Shard x and build the per-core scaled mask matrices.

    maskt[p, b*j + ji] = (p*j + ji < eff[b]) / max(eff[b], 1)
    """
    x = np.ascontiguousarray(np.asarray(x, dtype=np.float32))
    idx = np.asarray(start_padding_indices).astype(np.int64)
    j = s // P
    eff = np.where(idx == -1, s, idx).astype(np.int64)  # [B]
    scale = 1.0 / np.maximum(eff, 1).astype(np.float64)
    mask = (np.arange(s)[None, :] < eff[:, None]) * scale[:, None]  # [B, S] f64
    mask = mask.astype(np.float32)
    # [B, S] -> [B, P, j] (s-major within partition) -> cores pack [P, bl*j]
    mask_pj = mask.reshape(-1, P, j)  # [B, P, j]
    in_maps = []
    for c in range(n_cores):
        mb = mask_pj[c * bl : (c + 1) * bl]  # [bl, P, j]
        maskt = np.ascontiguousarray(mb.transpose(1, 0, 2).reshape(P, bl * j))
        in_maps.append(
            {
                "x": np.ascontiguousarray(x[c * bl : (c + 1) * bl]),
                "maskt": maskt,
            }
        )
    return in_maps


_CACHED_NC = None


def _get_nc():
    global _CACHED_NC
    if _CACHED_NC is None:
        _CACHED_NC = build_kernel()
    return _CACHED_NC


def run(x, start_padding_indices, trace=False):
    """Run on all 8 cores; returns (out [B, D] f32, BassKernelResults)."""
    nc = _get_nc()
    in_maps = make_host_inputs(x, start_padding_indices)
    res = bass_utils.run_bass_kernel_spmd(
        nc, in_maps, core_ids=list(range(N_CORES)), trace=trace
    )
    outs = [r["out"].reshape(BL, D) for r in res.results]
    return np.concatenate(outs, axis=0), res


def kernel(x, start_padding_indices):
    out, _ = run(x, start_padding_indices, trace=False)
    return out
